# revision 2
# baseline (speedup 1.0000x reference)
"""Trainium2 Bass kernel v2 for the 2-layer char-LSTM (B=64, T=512, H=1024).

Differences from v1:
- Two collectives per wave (one per layer), each issued as soon as that
  layer's h-chunk is ready, so the AllGather latency is covered by the other
  layer's matmuls instead of serializing at the end of the wave.
- Transposed cell math: after tanh(z), the four gate blocks are PE-transposed
  to (hid, batch) layout; all gate/cell ops then run on [128, 64] tiles
  (full lanes, shorter chain) and h emerges directly in the gather layout
  (no per-cell transpose at the end).
- PE program order [z1 | z2 | tr1 | proj | tr2] keeps the tensor engine busy
  while activations/DVE work and collectives fly.
"""
import sys

sys.path.insert(0, "/opt/trn_rl_repo")

import numpy as np
from concourse import bacc, tile, mybir
from concourse.bass_utils import run_bass_kernel_spmd

B, T, H, V, NC = 64, 512, 1024, 256, 8
KT = H // 128            # 8 contraction tiles of 128
HC = H // NC             # 128 hidden dims per core
GC = 4 * H // NC         # 512 gate cols per core
CH = 16                  # steps per one-hot chunk

DT = mybir.dt.float32
DTR = mybir.dt.float32r
AF = mybir.ActivationFunctionType
ALU = mybir.AluOpType


def r(ap):
    return ap.bitcast(DTR)


def build_nc(t_steps=T, reps=1, comm="split2"):
    nc = bacc.Bacc(None, target_bir_lowering=False, num_devices=NC)

    p_embed = nc.declare_dram_parameter("embed", [V, H], DTR, isOutput=False)
    p_wi0 = nc.declare_dram_parameter("wi0", [128, KT * GC], DTR, isOutput=False)
    p_wh0 = nc.declare_dram_parameter("wh0", [128, KT * GC], DTR, isOutput=False)
    p_wi1 = nc.declare_dram_parameter("wi1", [128, KT * GC], DTR, isOutput=False)
    p_wh1 = nc.declare_dram_parameter("wh1", [128, KT * GC], DTR, isOutput=False)
    p_b0 = nc.declare_dram_parameter("b0", [1, GC], DTR, isOutput=False)
    p_b1 = nc.declare_dram_parameter("b1", [1, GC], DTR, isOutput=False)
    p_wproj = nc.declare_dram_parameter("wproj", [128, KT * V], DTR, isOutput=False)
    p_oh = nc.declare_dram_parameter(
        "onehot", [2, 128, t_steps * B], DTR, isOutput=False
    )
    p_out = nc.declare_dram_parameter("out", [B, t_steps, V], DT, isOutput=True)

    c_ident = nc.inline_tensor(np.eye(128, dtype=np.float32), name="ident")
    c_ones = nc.inline_tensor(np.ones((1, B), dtype=np.float32), name="ones")

    from contextlib import ExitStack

    with tile.TileContext(nc) as tc, ExitStack() as stack:
        wp = stack.enter_context(tc.tile_pool(name="weights", bufs=1))
        wh0_sb = wp.tile([128, KT * GC], DTR, tag="wh0")
        wi1_sb = wp.tile([128, KT * GC], DTR, tag="wi1")
        wh1_sb = wp.tile([128, KT * GC], DTR, tag="wh1")
        wproj_sb = wp.tile([128, KT * V], DTR, tag="wproj")
        wie_sb = wp.tile([128, 2 * GC], DTR, tag="wie")
        b1_sb = wp.tile([1, GC], DTR, tag="b1")
        b0_sb = wp.tile([1, GC], DTR, tag="b0")
        ident_sb = wp.tile([128, 128], DTR, tag="ident")
        ones_sb = wp.tile([1, B], DTR, tag="ones")

        nc.sync.dma_start(wh0_sb[:], p_wh0[:])
        nc.sync.dma_start(wi1_sb[:], p_wi1[:])
        nc.sync.dma_start(wh1_sb[:], p_wh1[:])
        nc.sync.dma_start(wproj_sb[:], p_wproj[:])
        nc.sync.dma_start(b1_sb[:], p_b1[:])
        nc.sync.dma_start(b0_sb[:], p_b0[:])
        nc.gpsimd.dma_start(ident_sb[:], c_ident[:])
        nc.gpsimd.dma_start(ones_sb[:], c_ones[:])

        # ---- prologue: Wie = embed @ Wi0_c + b0_c, via on-device transpose
        with (
            tc.tile_pool(name="prolog", bufs=1) as pp,
            tc.tile_pool(name="prolog_ps", bufs=1, space="PSUM") as pps,
        ):
            wi0_sb = pp.tile([128, KT * GC], DTR, tag="wi0")
            em_sb = pp.tile([128, 2 * H], DTR, tag="em")  # vocab halves
            emt_sb = pp.tile([128, KT * V], DTR, tag="emt")
            nc.sync.dma_start(wi0_sb[:], p_wi0[:])
            nc.sync.dma_start(em_sb[:, 0:H], p_embed[0:128, :])
            nc.sync.dma_start(em_sb[:, H : 2 * H], p_embed[128:V, :])
            for k in range(KT):
                for vh in range(2):
                    pt = pps.tile([128, 128], DTR, tag="ptr")
                    nc.tensor.transpose(
                        r(pt[:]),
                        r(em_sb[:, vh * H + k * 128 : vh * H + (k + 1) * 128]),
                        r(ident_sb[:]),
                    )
                    nc.vector.tensor_copy(
                        emt_sb[:, k * V + vh * 128 : k * V + (vh + 1) * 128], pt[:]
                    )
            for m in range(2):
                ps = pps.tile([128, GC], DT, tag="pwie")
                for k in range(KT):
                    nc.tensor.matmul(
                        ps[:],
                        r(emt_sb[:, k * V + m * 128 : k * V + (m + 1) * 128]),
                        r(wi0_sb[:, k * GC : (k + 1) * GC]),
                        start=(k == 0),
                        stop=(k == KT - 1),
                    )
                nc.vector.tensor_copy(wie_sb[:, m * GC : (m + 1) * GC], ps[:])

        # ---- main loop pools
        hT = stack.enter_context(tc.tile_pool(name="hfull", bufs=3))
        cst = stack.enter_context(tc.tile_pool(name="cstate", bufs=3))
        oh = stack.enter_context(tc.tile_pool(name="onehot", bufs=2))
        gp = stack.enter_context(tc.tile_pool(name="gates", bufs=3))
        tp = stack.enter_context(tc.tile_pool(name="tmp", bufs=4))
        dr = stack.enter_context(tc.tile_pool(name="dram", bufs=6, space="DRAM"))
        zp = stack.enter_context(tc.tile_pool(name="zpsum", bufs=1, space="PSUM"))
        gps = stack.enter_context(tc.tile_pool(name="gapsum", bufs=1, space="PSUM"))
        pps2 = stack.enter_context(tc.tile_pool(name="ppsum", bufs=1, space="PSUM"))

        def gather(hc, tagpfx, old=None):
            """AllGather own (128, B) chunk -> (128, KT*B) k-tile layout."""
            agin = dr.tile([128, B], DTR, tag=tagpfx + "agin")
            agout = dr.tile([KT * 128, B], DTR, tag=tagpfx + "agout")
            nc.sync.dma_start(agin[:], hc[:])
            if comm == "nocomm":
                return old
            if comm != "local":
                nc.gpsimd.collective_compute(
                    "AllGather",
                    ALU.bypass,
                    replica_groups=[list(range(NC))],
                    ins=[agin[:].opt()],
                    outs=[agout[:].opt()],
                )
            else:
                nc.sync.dma_start(agout[0:128, :], agin[:])
            nxt = hT.tile([128, KT * B], DTR, tag=tagpfx + "hfull")
            src = agout[:].rearrange("(k p) j -> p k j", k=KT, p=128, j=B)
            dst = nxt[:].rearrange("p (k j) -> p k j", k=KT, j=B)
            nc.sync.dma_start(dst, src)
            return nxt

        def cell_tr(z, ga, c_prev, tagpfx):
            """Transposed cell: tanh(z) [64,512] -> 4 PE transposes ->
            (hid, batch) gate math. Returns (hT_sb [128,B], c_new [128,B])."""
            gaT = gps.tile([128, 4 * B], DT, tag=tagpfx + "gaT")
            for q in range(4):  # i, f, o, g
                nc.tensor.transpose(
                    r(gaT[:, q * B : (q + 1) * B]),
                    ga[:, q * 128 : (q + 1) * 128],
                    r(ident_sb[0:B, 0:B]),
                )
            sg = gp.tile([128, 3 * B], DT, tag=tagpfx + "sg")
            nc.vector.tensor_scalar(
                sg[:], gaT[:, 0 : 3 * B], 0.5, 0.5, ALU.mult, ALU.add
            )
            ig = tp.tile([128, B], DT, tag=tagpfx + "ig")
            nc.vector.tensor_tensor(ig[:], sg[:, 0:B], gaT[:, 3 * B : 4 * B], ALU.mult)
            cf = tp.tile([128, B], DT, tag=tagpfx + "cf")
            nc.vector.tensor_tensor(cf[:], c_prev[:], sg[:, B : 2 * B], ALU.mult)
            c_new = cst.tile([128, B], DT, tag=tagpfx + "c")
            nc.vector.tensor_tensor(c_new[:], ig[:], cf[:], ALU.add)
            th = tp.tile([128, B], DT, tag=tagpfx + "th")
            nc.scalar.activation(th[:], c_new[:], AF.Tanh)
            hc = tp.tile([128, B], DTR, tag=tagpfx + "h")
            nc.vector.tensor_tensor(hc[:], sg[:, 2 * B : 3 * B], th[:], ALU.mult)
            return hc, c_new

        for _ in range(reps):
            fused = comm == "fused"
            if fused:
                hbt = hT.tile([128, 2 * KT * B], DTR, tag="hbt")
                nc.vector.memset(hbt[:].bitcast(DT), 0.0)
                zsb = wp.tile([128, B], DTR, tag="zsb")
                nc.vector.memset(zsb[:].bitcast(DT), 0.0)

                def h1sl(k):
                    return hbt[:, (2 * k) * B : (2 * k + 1) * B]

                def h2sl(k):
                    return hbt[:, (2 * k + 1) * B : (2 * k + 2) * B]

                def fused_gather(h1c, h2c):
                    agin = dr.tile([2 * 128, B], DTR, tag="fagin")
                    agout = dr.tile([2 * KT * 128, B], DTR, tag="fagout")
                    nc.sync.dma_start(agin[0:128, :], h1c[:])
                    nc.sync.dma_start(agin[128:256, :], h2c[:])
                    nc.gpsimd.collective_compute(
                        "AllGather",
                        ALU.bypass,
                        replica_groups=[list(range(NC))],
                        ins=[agin[:].opt()],
                        outs=[agout[:].opt()],
                    )
                    nxt = hT.tile([128, 2 * KT * B], DTR, tag="hbt")
                    src = agout[:].rearrange(
                        "(s h p) j -> p s h j", s=KT, h=2, p=128, j=B
                    )
                    dst = nxt[:].rearrange("p (s h j) -> p s h j", s=KT, h=2, j=B)
                    nc.sync.dma_start(dst, src)
                    return nxt
            else:
                h1f = hT.tile([128, KT * B], DTR, tag="1hfull")
                h2f = hT.tile([128, KT * B], DTR, tag="2hfull")
                nc.vector.memset(h1f[:].bitcast(DT), 0.0)
                nc.vector.memset(h2f[:].bitcast(DT), 0.0)

                def h1sl(k):
                    return h1f[:, k * B : (k + 1) * B]

                def h2sl(k):
                    return h2f[:, k * B : (k + 1) * B]

            c1 = cst.tile([128, B], DT, tag="1c")
            c2 = cst.tile([128, B], DT, tag="2c")
            nc.vector.memset(c1[:], 0.0)
            nc.vector.memset(c2[:], 0.0)

            ohlo = ohhi = None
            for w in range(t_steps + 1):
                ga1 = ga2 = None
                z2 = None
                # ---- layer-1 matmuls (step w)
                if w < t_steps:
                    j = w % CH
                    if j == 0:
                        nch = min(CH, t_steps - w)
                        ohlo = oh.tile([128, CH * B], DTR, tag="ohlo")
                        ohhi = oh.tile([128, CH * B], DTR, tag="ohhi")
                        nc.sync.dma_start(
                            ohlo[:, 0 : nch * B], p_oh[0, :, w * B : (w + nch) * B]
                        )
                        nc.sync.dma_start(
                            ohhi[:, 0 : nch * B], p_oh[1, :, w * B : (w + nch) * B]
                        )
                    z1 = zp.tile([64, GC], DT, tag="z1")
                    nc.tensor.matmul(
                        z1[:], r(ones_sb[:]), r(b0_sb[:]), start=True, stop=False
                    )
                    nc.tensor.matmul(
                        z1[:], r(ohlo[:, j * B : (j + 1) * B]), r(wie_sb[:, 0:GC]),
                        start=False, stop=False,
                    )
                    nc.tensor.matmul(
                        z1[:], r(ohhi[:, j * B : (j + 1) * B]),
                        r(wie_sb[:, GC : 2 * GC]),
                        start=False, stop=False,
                    )
                    for k in range(KT):
                        nc.tensor.matmul(
                            z1[:],
                            r(h1sl(k)),
                            r(wh0_sb[:, k * GC : (k + 1) * GC]),
                            start=False,
                            stop=(k == KT - 1),
                        )
                    ga1 = gp.tile([64, GC], DTR, tag="1ga")
                    nc.scalar.activation(ga1[:], z1[:], AF.Tanh)
                # ---- layer-2 matmuls (step w-1)
                if w >= 1:
                    z2 = zp.tile([64, GC], DT, tag="z2")
                    nc.tensor.matmul(
                        z2[:], r(ones_sb[:]), r(b1_sb[:]), start=True, stop=False
                    )
                    for k in range(KT):
                        nc.tensor.matmul(
                            z2[:],
                            r(h1sl(k)),
                            r(wi1_sb[:, k * GC : (k + 1) * GC]),
                            start=False,
                            stop=False,
                        )
                    for k in range(KT):
                        nc.tensor.matmul(
                            z2[:],
                            r(h2sl(k)),
                            r(wh1_sb[:, k * GC : (k + 1) * GC]),
                            start=False,
                            stop=(k == KT - 1),
                        )
                # ---- cell 1 -> gather h1(w) as early as possible
                h1c = None
                if w < t_steps:
                    h1c, c1 = cell_tr(z1, ga1, c1, "1")
                    if not fused:
                        h1f_new = gather(h1c, "1", h1f)
                # ---- projection (step w-2) on current h2f (= h2(w-2) full),
                # emitted before this wave's h2f reassignment; covers tanh2
                if w >= 2:
                    pj = pps2.tile([64, V], DT, tag="pj")
                    for k in range(KT):
                        nc.tensor.matmul(
                            pj[:],
                            h2sl(k),
                            wproj_sb[:, k * V : (k + 1) * V],
                            start=(k == 0),
                            stop=(k == KT - 1),
                        )
                    lo = tp.tile([64, V], DT, tag="lo")
                    nc.vector.tensor_copy(lo[:], pj[:])
                    nc.sync.dma_start(p_out[:, w - 2, :], lo[:])
                # ---- cell 2 -> gather h2(w-1)
                h2c = None
                if w >= 1:
                    ga2 = gp.tile([64, GC], DTR, tag="2ga")
                    nc.scalar.activation(ga2[:], z2[:], AF.Tanh)
                    h2c, c2 = cell_tr(z2, ga2, c2, "2")
                    if not fused:
                        h2f = gather(h2c, "2", h2f)
                if fused:
                    hbt = fused_gather(
                        h1c if h1c is not None else zsb,
                        h2c if h2c is not None else zsb,
                    )
                elif w < t_steps:
                    h1f = h1f_new
            # final projection (step t_steps-1)
            pj = pps2.tile([64, V], DT, tag="pj")
            for k in range(KT):
                nc.tensor.matmul(
                    pj[:],
                    h2sl(k),
                    wproj_sb[:, k * V : (k + 1) * V],
                    start=(k == 0),
                    stop=(k == KT - 1),
                )
            lo = tp.tile([64, V], DT, tag="lo")
            nc.vector.tensor_copy(lo[:], pj[:])
            nc.sync.dma_start(p_out[:, t_steps - 1, :], lo[:])

    nc.compile()
    return nc


def prep_inputs(idx, embed, Wi, Wh, b, Wproj, t_steps=T, comm="split2"):
    """Host-side sharding/layout. Returns per-core in_maps."""
    order = [0, 1, 3, 2]  # i, f, o, g
    sc = np.concatenate([np.full(384, 0.5, np.float32), np.ones(128, np.float32)])
    rev2 = comm == "split2r"

    def mov(a):  # (1024, N) -> (128, 8*N) k-tile moving layout
        return np.ascontiguousarray(
            a.reshape(KT, 128, -1).transpose(1, 0, 2).reshape(128, -1)
        )

    def mov_r(a):  # reversed k-tile order (for reversed-group AllGather)
        return np.ascontiguousarray(
            a.reshape(KT, 128, -1)[::-1].transpose(1, 0, 2).reshape(128, -1)
        )

    idxf = idx[:, :t_steps].T.reshape(-1)  # (T*B,) t-major
    onehot = (
        (idxf[None, :] == np.arange(V, dtype=idxf.dtype)[:, None])
        .astype(np.float32)
        .reshape(2, 128, t_steps * B)
    )
    wproj = (mov_r if rev2 else mov)(Wproj)
    in_maps = []
    for c in range(NC):
        cols = np.concatenate(
            [np.arange(q * H + c * HC, q * H + (c + 1) * HC) for q in order]
        )
        m = {
            "embed": np.ascontiguousarray(embed),
            "wi0": mov(Wi[0][:, cols] * sc),
            "wh0": mov(Wh[0][:, cols] * sc),
            "wi1": mov(Wi[1][:, cols] * sc),
            "wh1": (mov_r if rev2 else mov)(Wh[1][:, cols] * sc),
            "b0": np.ascontiguousarray(b[0][cols] * sc).reshape(1, GC),
            "b1": np.ascontiguousarray(b[1][cols] * sc).reshape(1, GC),
            "wproj": wproj,
            "onehot": onehot,
        }
        in_maps.append({k: v.astype(v.dtype, copy=False) for k, v in m.items()})
    return in_maps


_NC_CACHE = {}


def _get_nc(t_steps, reps, comm="split2"):
    key = (t_steps, reps, comm)
    if key not in _NC_CACHE:
        _NC_CACHE[key] = build_nc(t_steps, reps, comm)
    return _NC_CACHE[key]


def run(idx, embed, Wi, Wh, b, Wproj, t_steps=T, reps=1, comm="split2"):
    nc = _get_nc(t_steps, reps, comm)
    in_maps = prep_inputs(idx, embed, Wi, Wh, b, Wproj, t_steps, comm=comm)
    res = run_bass_kernel_spmd(nc, in_maps, core_ids=list(range(NC)))
    return res.results[0]["out"]


def kernel(idx, embed, Wi, Wh, b, Wproj):
    out = run(
        np.asarray(idx), np.asarray(embed), np.asarray(Wi), np.asarray(Wh),
        np.asarray(b), np.asarray(Wproj),
    )
    return np.asarray(out, dtype=np.float32)


# revision 3
# speedup vs baseline: 1.0674x; 1.0674x over previous
"""Trainium2 Bass kernel v2 for the 2-layer char-LSTM (B=64, T=512, H=1024).

Differences from v1:
- Two collectives per wave (one per layer), each issued as soon as that
  layer's h-chunk is ready, so the AllGather latency is covered by the other
  layer's matmuls instead of serializing at the end of the wave.
- Transposed cell math: after tanh(z), the four gate blocks are PE-transposed
  to (hid, batch) layout; all gate/cell ops then run on [128, 64] tiles
  (full lanes, shorter chain) and h emerges directly in the gather layout
  (no per-cell transpose at the end).
- PE program order [z1 | z2 | tr1 | proj | tr2] keeps the tensor engine busy
  while activations/DVE work and collectives fly.
"""
import sys

sys.path.insert(0, "/opt/trn_rl_repo")

import numpy as np
from concourse import bacc, tile, mybir
from concourse.bass_utils import run_bass_kernel_spmd

B, T, H, V, NC = 64, 512, 1024, 256, 8
KT = H // 128            # 8 contraction tiles of 128
HC = H // NC             # 128 hidden dims per core
GC = 4 * H // NC         # 512 gate cols per core
CH = 16                  # steps per one-hot chunk

DT = mybir.dt.float32
DTR = mybir.dt.float32r
AF = mybir.ActivationFunctionType
ALU = mybir.AluOpType


def r(ap):
    return ap.bitcast(DTR)


def build_nc(t_steps=T, reps=1, comm="split2"):
    nc = bacc.Bacc(None, target_bir_lowering=False, num_devices=NC)

    p_embed = nc.declare_dram_parameter("embed", [V, H], DTR, isOutput=False)
    p_wi0 = nc.declare_dram_parameter("wi0", [128, KT * GC], DTR, isOutput=False)
    p_wh0 = nc.declare_dram_parameter("wh0", [128, KT * GC], DTR, isOutput=False)
    p_wi1 = nc.declare_dram_parameter("wi1", [128, KT * GC], DTR, isOutput=False)
    p_wh1 = nc.declare_dram_parameter("wh1", [128, KT * GC], DTR, isOutput=False)
    p_b0 = nc.declare_dram_parameter("b0", [1, GC], DTR, isOutput=False)
    p_b1 = nc.declare_dram_parameter("b1", [1, GC], DTR, isOutput=False)
    p_wproj = nc.declare_dram_parameter("wproj", [128, KT * V], DTR, isOutput=False)
    p_oh = nc.declare_dram_parameter(
        "onehot", [2, 128, t_steps * B], DTR, isOutput=False
    )
    p_out = nc.declare_dram_parameter("out", [B, t_steps, V], DT, isOutput=True)

    c_ident = nc.inline_tensor(np.eye(128, dtype=np.float32), name="ident")
    c_ones = nc.inline_tensor(np.ones((1, B), dtype=np.float32), name="ones")

    from contextlib import ExitStack

    with tile.TileContext(nc) as tc, ExitStack() as stack:
        wp = stack.enter_context(tc.tile_pool(name="weights", bufs=1))
        wh0_sb = wp.tile([128, KT * GC], DTR, tag="wh0")
        wi1_sb = wp.tile([128, KT * GC], DTR, tag="wi1")
        wh1_sb = wp.tile([128, KT * GC], DTR, tag="wh1")
        wproj_sb = wp.tile([128, KT * V], DTR, tag="wproj")
        wie_sb = wp.tile([128, 2 * GC], DTR, tag="wie")
        b1_sb = wp.tile([1, GC], DTR, tag="b1")
        b0_sb = wp.tile([1, GC], DTR, tag="b0")
        ident_sb = wp.tile([128, 128], DTR, tag="ident")
        ones_sb = wp.tile([1, B], DTR, tag="ones")

        nc.sync.dma_start(wh0_sb[:], p_wh0[:])
        nc.sync.dma_start(wi1_sb[:], p_wi1[:])
        nc.sync.dma_start(wh1_sb[:], p_wh1[:])
        nc.sync.dma_start(wproj_sb[:], p_wproj[:])
        nc.sync.dma_start(b1_sb[:], p_b1[:])
        nc.sync.dma_start(b0_sb[:], p_b0[:])
        nc.gpsimd.dma_start(ident_sb[:], c_ident[:])
        nc.gpsimd.dma_start(ones_sb[:], c_ones[:])

        # ---- prologue: Wie = embed @ Wi0_c + b0_c, via on-device transpose
        with (
            tc.tile_pool(name="prolog", bufs=1) as pp,
            tc.tile_pool(name="prolog_ps", bufs=1, space="PSUM") as pps,
        ):
            wi0_sb = pp.tile([128, KT * GC], DTR, tag="wi0")
            em_sb = pp.tile([128, 2 * H], DTR, tag="em")  # vocab halves
            emt_sb = pp.tile([128, KT * V], DTR, tag="emt")
            nc.sync.dma_start(wi0_sb[:], p_wi0[:])
            nc.sync.dma_start(em_sb[:, 0:H], p_embed[0:128, :])
            nc.sync.dma_start(em_sb[:, H : 2 * H], p_embed[128:V, :])
            for k in range(KT):
                for vh in range(2):
                    pt = pps.tile([128, 128], DTR, tag="ptr")
                    nc.tensor.transpose(
                        r(pt[:]),
                        r(em_sb[:, vh * H + k * 128 : vh * H + (k + 1) * 128]),
                        r(ident_sb[:]),
                    )
                    nc.vector.tensor_copy(
                        emt_sb[:, k * V + vh * 128 : k * V + (vh + 1) * 128], pt[:]
                    )
            for m in range(2):
                ps = pps.tile([128, GC], DT, tag="pwie")
                for k in range(KT):
                    nc.tensor.matmul(
                        ps[:],
                        r(emt_sb[:, k * V + m * 128 : k * V + (m + 1) * 128]),
                        r(wi0_sb[:, k * GC : (k + 1) * GC]),
                        start=(k == 0),
                        stop=(k == KT - 1),
                    )
                nc.vector.tensor_copy(wie_sb[:, m * GC : (m + 1) * GC], ps[:])

        # ---- main loop pools
        hT = stack.enter_context(tc.tile_pool(name="hfull", bufs=3))
        cst = stack.enter_context(tc.tile_pool(name="cstate", bufs=3))
        oh = stack.enter_context(tc.tile_pool(name="onehot", bufs=2))
        gp = stack.enter_context(tc.tile_pool(name="gates", bufs=3))
        tp = stack.enter_context(tc.tile_pool(name="tmp", bufs=4))
        dr = stack.enter_context(tc.tile_pool(name="dram", bufs=6, space="DRAM"))
        zp = stack.enter_context(tc.tile_pool(name="zpsum", bufs=1, space="PSUM"))
        gps = stack.enter_context(tc.tile_pool(name="gapsum", bufs=1, space="PSUM"))
        pps2 = stack.enter_context(tc.tile_pool(name="ppsum", bufs=1, space="PSUM"))

        def gather(hc, tagpfx, old=None):
            """AllGather own (128, B) chunk -> (128, KT*B) k-tile layout."""
            agin = dr.tile([128, B], DTR, tag=tagpfx + "agin")
            agout = dr.tile([KT * 128, B], DTR, tag=tagpfx + "agout")
            nc.sync.dma_start(agin[:], hc[:])
            if comm == "nocomm":
                return old
            if comm != "local":
                nc.gpsimd.collective_compute(
                    "AllGather",
                    ALU.bypass,
                    replica_groups=[list(range(NC))],
                    ins=[agin[:].opt()],
                    outs=[agout[:].opt()],
                )
            else:
                nc.sync.dma_start(agout[0:128, :], agin[:])
            nxt = hT.tile([128, KT * B], DTR, tag=tagpfx + "hfull")
            src = agout[:].rearrange("(k p) j -> p k j", k=KT, p=128, j=B)
            dst = nxt[:].rearrange("p (k j) -> p k j", k=KT, j=B)
            nc.sync.dma_start(dst, src)
            return nxt

        def cell_tr(z, ga, c_prev, tagpfx):
            """Transposed cell: tanh(z) [64,512] -> 4 PE transposes ->
            (hid, batch) gate math. Returns (hT_sb [128,B], c_new [128,B])."""
            gaT = gps.tile([128, 4 * B], DT, tag=tagpfx + "gaT")
            for q in range(4):  # i, f, o, g
                nc.tensor.transpose(
                    r(gaT[:, q * B : (q + 1) * B]),
                    ga[:, q * 128 : (q + 1) * 128],
                    r(ident_sb[0:B, 0:B]),
                )
            sg = gp.tile([128, 3 * B], DT, tag=tagpfx + "sg")
            nc.vector.tensor_scalar(
                sg[:], gaT[:, 0 : 3 * B], 0.5, 0.5, ALU.mult, ALU.add
            )
            ig = tp.tile([128, B], DT, tag=tagpfx + "ig")
            nc.vector.tensor_tensor(ig[:], sg[:, 0:B], gaT[:, 3 * B : 4 * B], ALU.mult)
            cf = tp.tile([128, B], DT, tag=tagpfx + "cf")
            nc.vector.tensor_tensor(cf[:], c_prev[:], sg[:, B : 2 * B], ALU.mult)
            c_new = cst.tile([128, B], DT, tag=tagpfx + "c")
            nc.vector.tensor_tensor(c_new[:], ig[:], cf[:], ALU.add)
            th = tp.tile([128, B], DT, tag=tagpfx + "th")
            nc.scalar.activation(th[:], c_new[:], AF.Tanh)
            hc = tp.tile([128, B], DTR, tag=tagpfx + "h")
            nc.vector.tensor_tensor(hc[:], sg[:, 2 * B : 3 * B], th[:], ALU.mult)
            return hc, c_new

        for _ in range(reps):
            fused = comm == "fused"
            if fused:
                hbt = hT.tile([128, 2 * KT * B], DTR, tag="hbt")
                nc.vector.memset(hbt[:].bitcast(DT), 0.0)
                zsb = wp.tile([128, B], DTR, tag="zsb")
                nc.vector.memset(zsb[:].bitcast(DT), 0.0)

                def h1sl(k):
                    return hbt[:, (2 * k) * B : (2 * k + 1) * B]

                def h2sl(k):
                    return hbt[:, (2 * k + 1) * B : (2 * k + 2) * B]

                def fused_gather(h1c, h2c):
                    agin = dr.tile([2 * 128, B], DTR, tag="fagin")
                    agout = dr.tile([2 * KT * 128, B], DTR, tag="fagout")
                    nc.sync.dma_start(agin[0:128, :], h1c[:])
                    nc.sync.dma_start(agin[128:256, :], h2c[:])
                    nc.gpsimd.collective_compute(
                        "AllGather",
                        ALU.bypass,
                        replica_groups=[list(range(NC))],
                        ins=[agin[:].opt()],
                        outs=[agout[:].opt()],
                    )
                    nxt = hT.tile([128, 2 * KT * B], DTR, tag="hbt")
                    src = agout[:].rearrange(
                        "(s h p) j -> p s h j", s=KT, h=2, p=128, j=B
                    )
                    dst = nxt[:].rearrange("p (s h j) -> p s h j", s=KT, h=2, j=B)
                    nc.sync.dma_start(dst, src)
                    return nxt
            else:
                h1f = hT.tile([128, KT * B], DTR, tag="1hfull")
                h2f = hT.tile([128, KT * B], DTR, tag="2hfull")
                nc.vector.memset(h1f[:].bitcast(DT), 0.0)
                nc.vector.memset(h2f[:].bitcast(DT), 0.0)

                def h1sl(k):
                    return h1f[:, k * B : (k + 1) * B]

                def h2sl(k):
                    return h2f[:, k * B : (k + 1) * B]

            c1 = cst.tile([128, B], DT, tag="1c")
            c2 = cst.tile([128, B], DT, tag="2c")
            nc.vector.memset(c1[:], 0.0)
            nc.vector.memset(c2[:], 0.0)

            ohlo = ohhi = None
            for w in range(t_steps + 1):
                ga1 = ga2 = None
                z2 = None
                # ---- layer-1 matmuls (step w)
                if w < t_steps:
                    j = w % CH
                    if j == 0:
                        nch = min(CH, t_steps - w)
                        ohlo = oh.tile([128, CH * B], DTR, tag="ohlo")
                        ohhi = oh.tile([128, CH * B], DTR, tag="ohhi")
                        nc.sync.dma_start(
                            ohlo[:, 0 : nch * B], p_oh[0, :, w * B : (w + nch) * B]
                        )
                        nc.sync.dma_start(
                            ohhi[:, 0 : nch * B], p_oh[1, :, w * B : (w + nch) * B]
                        )
                    z1 = zp.tile([64, GC], DT, tag="z1")
                    nc.tensor.matmul(
                        z1[:], r(ones_sb[:]), r(b0_sb[:]), start=True, stop=False
                    )
                    nc.tensor.matmul(
                        z1[:], r(ohlo[:, j * B : (j + 1) * B]), r(wie_sb[:, 0:GC]),
                        start=False, stop=False,
                    )
                    nc.tensor.matmul(
                        z1[:], r(ohhi[:, j * B : (j + 1) * B]),
                        r(wie_sb[:, GC : 2 * GC]),
                        start=False, stop=False,
                    )
                    for k in range(KT):
                        nc.tensor.matmul(
                            z1[:],
                            r(h1sl(k)),
                            r(wh0_sb[:, k * GC : (k + 1) * GC]),
                            start=False,
                            stop=(k == KT - 1),
                        )
                    ga1 = gp.tile([64, GC], DTR, tag="1ga")
                    nc.scalar.activation(ga1[:, 0:256], z1[:, 0:256], AF.Tanh)
                    nc.scalar.activation(ga1[:, 256:GC], z1[:, 256:GC], AF.Tanh)
                # ---- layer-2 matmuls (step w-1)
                if w >= 1:
                    z2 = zp.tile([64, GC], DT, tag="z2")
                    nc.tensor.matmul(
                        z2[:], r(ones_sb[:]), r(b1_sb[:]), start=True, stop=False
                    )
                    for k in range(KT):
                        nc.tensor.matmul(
                            z2[:],
                            r(h1sl(k)),
                            r(wi1_sb[:, k * GC : (k + 1) * GC]),
                            start=False,
                            stop=False,
                        )
                    for k in range(KT):
                        nc.tensor.matmul(
                            z2[:],
                            r(h2sl(k)),
                            r(wh1_sb[:, k * GC : (k + 1) * GC]),
                            start=False,
                            stop=(k == KT - 1),
                        )
                # ---- cell 1 -> gather h1(w) as early as possible
                h1c = None
                if w < t_steps:
                    h1c, c1 = cell_tr(z1, ga1, c1, "1")
                    if not fused:
                        h1f_new = gather(h1c, "1", h1f)
                # ---- projection (step w-2) on current h2f (= h2(w-2) full),
                # emitted before this wave's h2f reassignment; covers tanh2
                if w >= 2:
                    pj = pps2.tile([64, V], DT, tag="pj")
                    for k in range(KT):
                        nc.tensor.matmul(
                            pj[:],
                            h2sl(k),
                            wproj_sb[:, k * V : (k + 1) * V],
                            start=(k == 0),
                            stop=(k == KT - 1),
                        )
                    lo = tp.tile([64, V], DT, tag="lo")
                    nc.vector.tensor_copy(lo[:], pj[:])
                    nc.sync.dma_start(p_out[:, w - 2, :], lo[:])
                # ---- cell 2 -> gather h2(w-1)
                h2c = None
                if w >= 1:
                    ga2 = gp.tile([64, GC], DTR, tag="2ga")
                    nc.scalar.activation(ga2[:, 0:256], z2[:, 0:256], AF.Tanh)
                    nc.scalar.activation(ga2[:, 256:GC], z2[:, 256:GC], AF.Tanh)
                    h2c, c2 = cell_tr(z2, ga2, c2, "2")
                    if not fused:
                        h2f = gather(h2c, "2", h2f)
                if fused:
                    hbt = fused_gather(
                        h1c if h1c is not None else zsb,
                        h2c if h2c is not None else zsb,
                    )
                elif w < t_steps:
                    h1f = h1f_new
            # final projection (step t_steps-1)
            pj = pps2.tile([64, V], DT, tag="pj")
            for k in range(KT):
                nc.tensor.matmul(
                    pj[:],
                    h2sl(k),
                    wproj_sb[:, k * V : (k + 1) * V],
                    start=(k == 0),
                    stop=(k == KT - 1),
                )
            lo = tp.tile([64, V], DT, tag="lo")
            nc.vector.tensor_copy(lo[:], pj[:])
            nc.sync.dma_start(p_out[:, t_steps - 1, :], lo[:])

    nc.compile()
    return nc


def prep_inputs(idx, embed, Wi, Wh, b, Wproj, t_steps=T, comm="split2"):
    """Host-side sharding/layout. Returns per-core in_maps."""
    order = [0, 1, 3, 2]  # i, f, o, g
    sc = np.concatenate([np.full(384, 0.5, np.float32), np.ones(128, np.float32)])
    rev2 = comm == "split2r"

    def mov(a):  # (1024, N) -> (128, 8*N) k-tile moving layout
        return np.ascontiguousarray(
            a.reshape(KT, 128, -1).transpose(1, 0, 2).reshape(128, -1)
        )

    def mov_r(a):  # reversed k-tile order (for reversed-group AllGather)
        return np.ascontiguousarray(
            a.reshape(KT, 128, -1)[::-1].transpose(1, 0, 2).reshape(128, -1)
        )

    idxf = idx[:, :t_steps].T.reshape(-1)  # (T*B,) t-major
    onehot = (
        (idxf[None, :] == np.arange(V, dtype=idxf.dtype)[:, None])
        .astype(np.float32)
        .reshape(2, 128, t_steps * B)
    )
    wproj = (mov_r if rev2 else mov)(Wproj)
    in_maps = []
    for c in range(NC):
        cols = np.concatenate(
            [np.arange(q * H + c * HC, q * H + (c + 1) * HC) for q in order]
        )
        m = {
            "embed": np.ascontiguousarray(embed),
            "wi0": mov(Wi[0][:, cols] * sc),
            "wh0": mov(Wh[0][:, cols] * sc),
            "wi1": mov(Wi[1][:, cols] * sc),
            "wh1": (mov_r if rev2 else mov)(Wh[1][:, cols] * sc),
            "b0": np.ascontiguousarray(b[0][cols] * sc).reshape(1, GC),
            "b1": np.ascontiguousarray(b[1][cols] * sc).reshape(1, GC),
            "wproj": wproj,
            "onehot": onehot,
        }
        in_maps.append({k: v.astype(v.dtype, copy=False) for k, v in m.items()})
    return in_maps


_NC_CACHE = {}


def _get_nc(t_steps, reps, comm="split2"):
    key = (t_steps, reps, comm)
    if key not in _NC_CACHE:
        _NC_CACHE[key] = build_nc(t_steps, reps, comm)
    return _NC_CACHE[key]


def run(idx, embed, Wi, Wh, b, Wproj, t_steps=T, reps=1, comm="split2"):
    nc = _get_nc(t_steps, reps, comm)
    in_maps = prep_inputs(idx, embed, Wi, Wh, b, Wproj, t_steps, comm=comm)
    res = run_bass_kernel_spmd(nc, in_maps, core_ids=list(range(NC)))
    return res.results[0]["out"]


def kernel(idx, embed, Wi, Wh, b, Wproj):
    out = run(
        np.asarray(idx), np.asarray(embed), np.asarray(Wi), np.asarray(Wh),
        np.asarray(b), np.asarray(Wproj),
    )
    return np.asarray(out, dtype=np.float32)
